# revision 6
# baseline (speedup 1.0000x reference)
"""Trainium2 Bass kernel for the distributed DCRNN (gnn_message_passing) problem.

Strategy: node-shard across 8 cores (dealt by in-degree rank so all cores share
one compiled grid geometry). All graph indirection is resolved HOST-side by
duplicating INPUT data per edge cell (pure index plumbing — no host arithmetic):

  - xdup[cell]  = x[src(cell)]           (bf16)
  - wdup[cell]  = src's full weight list (bf16, padded to K)

The device re-derives the per-edge scale on-chip (deg = reduce(wdup),
s = 1/deg), forms messages m = xdup * s (bf16, 2x DVE mode), and
segment-reduces per destination. Bulk data moves via regular strided HWDGE
DMAs at full bandwidth.

The A-direction (in-edges per dest) uses in-degree-sorted node order (pi);
the B-direction uses out-degree-sorted order (sigma) for tight tile widths,
and its result TxI is permuted sigma->pi once at node granularity via a
DRAM round-trip + SWDGE dma_gather on the otherwise-idle GpSimd engine,
overlapped under the A-direction stream.
"""

from contextlib import ExitStack

import ml_dtypes
import numpy as np

import concourse.bass as bass
import concourse.bacc as bacc
import concourse.mybir as mybir
import concourse.tile as tile
from concourse.masks import make_identity

P = 128
CH = 16
FILT = 64
CWMAX = 192    # grid columns per streamed chunk
NIMAX = 1024   # max descriptors per dma_gather call (SWDGE ring capacity)


# ---------------------------------------------------------------------------
# host-side preprocessing (index plumbing only; no reference arithmetic)
# ---------------------------------------------------------------------------

def chunk_plan(D, max_width=CWMAX):
    """Split tiles into chunks (contiguous tiles, bounded total width) and
    equal-width runs within each chunk: (t_lo, t_hi, off_lo, off_hi, runs)."""
    T = len(D)
    off = np.concatenate([[0], np.cumsum(D)]).astype(np.int64)
    chunks = []
    t = 0
    while t < T:
        t0 = t
        w = 0
        while t < T and (w + D[t] <= max_width or t == t0):
            w += D[t]
            t += 1
        runs = []
        r = t0
        while r < t:
            r0 = r
            while r < t and D[r] == D[r0]:
                r += 1
            runs.append((r0, r, int(D[r0])))
        chunks.append((t0, t, int(off[t0]), int(off[t]), runs))
    return chunks


def preprocess(x, edge_index, edge_weight, n_cores=8):
    N = x.shape[0]
    E = edge_index.shape[1]
    NPC = N // n_cores
    T = (NPC + P - 1) // P
    NL = P * T
    row = np.ascontiguousarray(edge_index[0]).astype(np.int64)
    col = np.ascontiguousarray(edge_index[1]).astype(np.int64)
    w = np.ascontiguousarray(edge_weight).astype(np.float32)

    cnt_in = np.bincount(col, minlength=N)
    cnt_out = np.bincount(row, minlength=N)

    # deal nodes to cores by global in-degree rank so per-tile degree profiles
    # match across cores (one compiled kernel; minimal tile-width padding)
    g_order = np.argsort(cnt_in, kind="stable")
    cores = np.empty(N, dtype=np.int64)
    cores[g_order] = np.arange(N) % n_cores

    def make_perm(cnt):
        perm = np.full((n_cores, NL), -1, dtype=np.int64)
        pos = np.empty(N, dtype=np.int64)
        for k in range(n_cores):
            nodes = np.where(cores == k)[0]
            order = np.argsort(cnt[nodes], kind="stable")
            perm[k, :NPC] = nodes[order]
            pos[nodes[order]] = np.arange(NPC)
        return perm, pos

    permA, posA = make_perm(cnt_in)   # pi: A-grid dest order, output order
    permB, posB = make_perm(cnt_out)  # sigma: B-grid dest order

    def tile_widths(perm, cnt):
        D = np.zeros(T, dtype=np.int64)
        for k in range(n_cores):
            c = np.where(perm[k] >= 0, cnt[np.maximum(perm[k], 0)], 0)
            D = np.maximum(D, c.reshape(T, P).max(axis=1))
        return np.maximum(D, 1)

    DA = tile_widths(permA, cnt_in)    # A-grid: in-edges per dest (dest = col)
    DB = tile_widths(permB, cnt_out)   # B-grid: out-edges per dest (dest = row)
    offA = np.concatenate([[0], np.cumsum(DA)]).astype(np.int64)
    offB = np.concatenate([[0], np.cumsum(DB)]).astype(np.int64)
    WA, WB = int(offA[-1]), int(offB[-1])

    def kpad(k):
        return 3 * ((int(k) + 2) // 3)
    KA = kpad(cnt_out.max())   # A scale = 1/deg_out(src)
    KB = kpad(cnt_in.max())    # B scale = 1/deg_in(src)

    # padded per-node weight lists (bf16)
    def weight_lists(key, K):
        wp = np.zeros((N, K), dtype=ml_dtypes.bfloat16)
        order = np.argsort(key, kind="stable")
        ks = key[order]
        start = np.concatenate([[0], np.cumsum(np.bincount(ks, minlength=N))])[ks]
        slot = np.arange(E) - start
        wp[ks, slot] = w[order].astype(ml_dtypes.bfloat16)
        return wp
    w_out_pad = weight_lists(row, KA)
    w_in_pad = weight_lists(col, KB)

    xbf = np.asarray(x, dtype=np.float32).astype(ml_dtypes.bfloat16)

    def build_dup(dest, src, pos, off, W, w_pad, K):
        xdup = np.zeros((n_cores, P, W, CH), dtype=ml_dtypes.bfloat16)
        wdup = np.zeros((n_cores, P, W, K), dtype=ml_dtypes.bfloat16)
        k_e = cores[dest]
        j_e = pos[dest]
        t_e, p_e = j_e // P, j_e % P
        order = np.argsort(dest, kind="stable")
        ds = dest[order]
        start = np.concatenate([[0], np.cumsum(np.bincount(ds, minlength=N))])[ds]
        s_e = np.empty(E, dtype=np.int64)
        s_e[order] = np.arange(E) - start
        wcol = off[t_e] + s_e
        xdup[k_e, p_e, wcol] = xbf[src]
        wdup[k_e, p_e, wcol] = w_pad[src]
        return xdup, wdup

    xdupA, wdupA = build_dup(col, row, posA, offA, WA, w_out_pad, KA)
    xdupB, wdupB = build_dup(row, col, posB, offB, WB, w_in_pad, KB)

    # sigma->pi permute index list for the TxI gather: pi cell (p, t) holds
    # node v = permA[k][t*P+p]; its TxIs DRAM row is (posB%P)*T + posB//P.
    # dma_gather linear index i -> output (i%128, i//128), so list order is
    # t-major / p-fast. Wrapped int16 layout: index i at [i%16, i//16],
    # replicated across the 8 Q7 cores (partition groups of 16).
    nip = NIMAX - NIMAX % P
    while NL % nip != 0:
        nip -= P
    ncall = NL // nip
    pad_sig_row = (NPC % P) * T + NPC // P
    permI16 = np.zeros((n_cores, P, NL // 16), dtype=np.int16)
    for k in range(n_cores):
        pk = permA[k]
        sig = np.full(NL, pad_sig_row, dtype=np.int64)
        validA = pk >= 0
        jB = posB[np.maximum(pk, 0)]
        sig[validA] = ((jB % P) * T + jB // P)[validA]
        # sig is indexed by pi j = t*P+p; reorder to i = t*128+p (identical)
        lst = sig.astype(np.int16)                      # [NL] in i order
        wrapped = np.concatenate(
            [lst[c * nip:(c + 1) * nip].reshape(-1, 16).T for c in range(ncall)],
            axis=1)                                     # [16, NL/16]
        permI16[k] = np.tile(wrapped, (8, 1))

    xT = np.zeros((n_cores, CH, NL), dtype=np.float32)
    for k in range(n_cores):
        pk = permA[k]
        valid = pk >= 0
        xg = np.zeros((NL, CH), dtype=np.float32)
        xg[valid] = np.asarray(x, dtype=np.float32)[pk[valid]]
        xT[k] = xg.T

    cfg = dict(
        N=N, E=E, NPC=NPC, T=T, NL=NL, WA=WA, WB=WB, KA=KA, KB=KB,
        n_cores=n_cores, chunksA=chunk_plan(DA), chunksB=chunk_plan(DB),
        nip=nip, ncall=ncall,
    )
    arrays = dict(
        xdupA=xdupA, wdupA=wdupA, xdupB=xdupB, wdupB=wdupB, xT=xT,
        permI16=permI16, permA=permA,
    )
    return cfg, arrays


def make_in_maps(cfg, arrays, w_z, b_z, w_h, b_h, lin_w, lin_b):
    """AT row layout: [x^T (0:16) | zeros (16:32) | TxO^T (32:48) | TxI^T (48:64)].
    Wcat rows match; rows 16:32 are zero (contraction-dim padding is free)."""
    n_cores = cfg["n_cores"]
    w_id0 = np.concatenate([w_z[0, 0, :CH], w_h[0, 0, :CH]], axis=1).astype(np.float32)
    w_id1 = np.concatenate([w_z[1, 0, :CH], w_h[1, 0, :CH]], axis=1).astype(np.float32)
    w_dif = np.concatenate(
        [np.concatenate([w_z[0, 1, :CH], w_h[0, 1, :CH]], axis=1),
         np.concatenate([w_z[1, 1, :CH], w_h[1, 1, :CH]], axis=1)],
        axis=0).astype(np.float32)
    bias = np.concatenate([b_z, b_h]).astype(np.float32).reshape(P, 1)
    in_maps = []
    for k in range(n_cores):
        in_maps.append({
            "xT": np.ascontiguousarray(arrays["xT"][k]),
            "xdupA": np.ascontiguousarray(arrays["xdupA"][k]),
            "wdupA": np.ascontiguousarray(arrays["wdupA"][k]),
            "xdupB": np.ascontiguousarray(arrays["xdupB"][k]),
            "wdupB": np.ascontiguousarray(arrays["wdupB"][k]),
            "permI16": np.ascontiguousarray(arrays["permI16"][k]),
            "w_id0": w_id0, "w_id1": w_id1, "w_dif": w_dif,
            "bias": bias,
            "lin_w": lin_w.astype(np.float32),
            "lin_b": lin_b.astype(np.float32).reshape(1, 1),
        })
    return in_maps


def postprocess(cfg, arrays, results):
    """results[k]['out'] is [1, NL]; scatter back to [N, 1] full output."""
    N, NL = cfg["N"], cfg["NL"]
    out = np.zeros((N, 1), dtype=np.float32)
    for k in range(cfg["n_cores"]):
        o = np.asarray(results[k]["out"]).reshape(NL)
        pk = arrays["permA"][k]
        valid = pk >= 0
        out[pk[valid], 0] = o[valid]
    return out


# ---------------------------------------------------------------------------
# device kernel
# ---------------------------------------------------------------------------

def build_kernel(cfg, debug=False):
    T, NL, WA, WB = cfg["T"], cfg["NL"], cfg["WA"], cfg["WB"]
    KA, KB = cfg["KA"], cfg["KB"]
    nip, ncall = cfg["nip"], cfg["ncall"]
    f32 = mybir.dt.float32
    bf16 = mybir.dt.bfloat16
    i16 = mybir.dt.int16

    nc = bacc.Bacc()

    xT_p = nc.declare_dram_parameter("xT", [CH, NL], f32, isOutput=False)
    xdupA_p = nc.declare_dram_parameter("xdupA", [P, WA, CH], bf16, isOutput=False)
    wdupA_p = nc.declare_dram_parameter("wdupA", [P, WA, KA], bf16, isOutput=False)
    xdupB_p = nc.declare_dram_parameter("xdupB", [P, WB, CH], bf16, isOutput=False)
    wdupB_p = nc.declare_dram_parameter("wdupB", [P, WB, KB], bf16, isOutput=False)
    permI_p = nc.declare_dram_parameter("permI16", [P, NL // 16], i16, isOutput=False)
    w_id0_p = nc.declare_dram_parameter("w_id0", [CH, P], f32, isOutput=False)
    w_id1_p = nc.declare_dram_parameter("w_id1", [CH, P], f32, isOutput=False)
    w_dif_p = nc.declare_dram_parameter("w_dif", [2 * CH, P], f32, isOutput=False)
    bias_p = nc.declare_dram_parameter("bias", [P, 1], f32, isOutput=False)
    lin_w_p = nc.declare_dram_parameter("lin_w", [FILT, 1], f32, isOutput=False)
    lin_b_p = nc.declare_dram_parameter("lin_b", [1, 1], f32, isOutput=False)
    out_p = nc.declare_dram_parameter("out", [1, NL], f32, isOutput=True)

    # TxI bounce: 256B rows (64 f32), channels 0:16 used
    txis_d = nc.dram_tensor("txis", [NL, 64], f32)

    with ExitStack() as ctx:
        tc = ctx.enter_context(tile.TileContext(nc))
        persist = ctx.enter_context(tc.tile_pool(name="persist", bufs=1))
        wpool = ctx.enter_context(tc.tile_pool(name="wpool", bufs=2))
        xpool = ctx.enter_context(tc.tile_pool(name="xpool", bufs=2))
        spool = ctx.enter_context(tc.tile_pool(name="spool", bufs=2))
        mpool = ctx.enter_context(tc.tile_pool(name="mpool", bufs=2))
        work = ctx.enter_context(tc.tile_pool(name="work", bufs=2))
        psum = ctx.enter_context(tc.tile_pool(name="psum", bufs=2, space="PSUM"))
        psum_pre = ctx.enter_context(tc.tile_pool(name="psum_pre", bufs=2, space="PSUM"))

        # ---- persistent tiles & input DMAs ----
        AT = persist.tile([FILT, NL], f32)
        TxC = persist.tile([P, T, 2 * CH], f32)
        TxIs = persist.tile([P, T, CH], f32)
        G2 = persist.tile([P, T, 64], f32)
        permI_t = persist.tile([P, NL // 16], i16)
        Wcat = persist.tile([FILT, P], f32)
        w_id0_t = persist.tile([CH, P], f32)
        w_id1_t = persist.tile([CH, P], f32)
        bias_t = persist.tile([P, 1], f32)
        bias_h = persist.tile([P, 1], f32)
        lin_w_t = persist.tile([FILT, 1], f32)
        lin_b_t = persist.tile([1, 1], f32)
        ident = persist.tile([P, P], f32)

        nc.vector.memset(AT[0:2 * CH, :], 0.0)
        nc.vector.memset(Wcat[0:2 * CH, :], 0.0)
        nc.sync.dma_start(out=AT[0:CH, :], in_=xT_p[:])
        nc.sync.dma_start(out=permI_t[:], in_=permI_p[:])
        nc.sync.dma_start(out=w_id0_t[:], in_=w_id0_p[:])
        nc.sync.dma_start(out=w_id1_t[:], in_=w_id1_p[:])
        nc.sync.dma_start(out=Wcat[2 * CH:4 * CH, :], in_=w_dif_p[:])
        nc.sync.dma_start(out=bias_t[:], in_=bias_p[:])
        nc.sync.dma_start(out=lin_w_t[:], in_=lin_w_p[:])
        nc.sync.dma_start(out=lin_b_t[:], in_=lin_b_p[:])
        make_identity(nc, ident[:])

        nc.vector.tensor_add(out=Wcat[0:CH, :], in0=w_id0_t[:], in1=w_id1_t[:])
        # bias halves: Z-part scaled by 0.5 for the tanh-based sigmoid
        nc.vector.tensor_scalar_mul(out=bias_h[0:FILT, :], in0=bias_t[0:FILT, :],
                                    scalar1=0.5)
        nc.vector.tensor_copy(out=bias_h[FILT:P, :], in_=bias_t[FILT:P, :])

        # ---- streamed message passing ----
        # per chunk: load [P, Wc, K] weights + [P, Wc, CH] features; on-chip
        # deg = reduce(w) (two bf16 2x-mode folds K->K/3, then reduce),
        # s = 1/deg, m = x * s (bf16), segment-reduce into the target.
        # B first: its sigma->pi permute then overlaps the A stream.
        ctx.enter_context(nc.allow_low_precision(
            reason="bf16 edge pipeline; rel tolerance 2e-2 vs bf16 ~4e-3"))

        def stream(xdup_p, wdup_p, Kd, chunks, tx_out, ch0):
            K3 = Kd // 3
            for (t0, t1, o0, o1, runs) in chunks:
                Wc = o1 - o0
                wd = wpool.tile([P, CWMAX, Kd], bf16, tag="wd")
                nc.sync.dma_start(out=wd[:, 0:Wc, :], in_=wdup_p[:, o0:o1, :])
                xd = xpool.tile([P, CWMAX, CH], bf16, tag="xd")
                nc.scalar.dma_start(out=xd[:, 0:Wc, :], in_=xdup_p[:, o0:o1, :])
                fold = mpool.tile([P, CWMAX, K3], bf16, tag="fold")
                nc.vector.tensor_tensor(out=fold[:, 0:Wc, :],
                                        in0=wd[:, 0:Wc, 0:K3],
                                        in1=wd[:, 0:Wc, K3:2 * K3],
                                        op=mybir.AluOpType.add)
                nc.vector.tensor_tensor(out=fold[:, 0:Wc, :],
                                        in0=fold[:, 0:Wc, :],
                                        in1=wd[:, 0:Wc, 2 * K3:Kd],
                                        op=mybir.AluOpType.add)
                s = spool.tile([P, CWMAX], f32, tag="s")
                nc.vector.tensor_reduce(out=s[:, 0:Wc], in_=fold[:, 0:Wc, :],
                                        axis=mybir.AxisListType.X,
                                        op=mybir.AluOpType.add)
                nc.vector.tensor_scalar_max(out=s[:, 0:Wc], in0=s[:, 0:Wc],
                                            scalar1=1e-30)
                sb = spool.tile([P, CWMAX], bf16, tag="sb")
                nc.vector.reciprocal(out=sb[:, 0:Wc], in_=s[:, 0:Wc])
                m = mpool.tile([P, CWMAX, CH], bf16, tag="m")
                nc.vector.tensor_tensor(out=m[:, 0:Wc, :], in0=xd[:, 0:Wc, :],
                                        in1=sb[:, 0:Wc].to_broadcast([P, Wc, CH]),
                                        op=mybir.AluOpType.mult)
                ro = 0
                for (r0, r1, D) in runs:
                    nt = r1 - r0
                    nc.vector.tensor_reduce(
                        out=tx_out[:, r0:r1, ch0:ch0 + CH],
                        in_=m[:, ro:ro + nt * D, :].rearrange(
                            "p (t d) c -> p t c d", t=nt),
                        axis=mybir.AxisListType.X, op=mybir.AluOpType.add)
                    ro += nt * D

        stream(xdupB_p, wdupB_p, KB, cfg["chunksB"], TxIs, 0)

        # sigma->pi permute of TxIs via DRAM round-trip + SWDGE gather
        # (runs on GpSimd, overlapped under the A stream below)
        nc.sync.dma_start(out=txis_d[:, 0:CH], in_=TxIs[:])
        for c in range(ncall):
            nc.gpsimd.dma_gather(
                G2[:, c * (nip // P):(c + 1) * (nip // P), :], txis_d[:],
                permI_t[:, c * (nip // 16):(c + 1) * (nip // 16)],
                nip, nip, 64)
        nc.scalar.copy(out=TxC[:, :, CH:2 * CH], in_=G2[:, :, 0:CH])

        stream(xdupA_p, wdupA_p, KA, cfg["chunksA"], TxC, 0)

        # ---- transposes into AT rows 32:64 ----
        # 4 tiles per transpose: out rows 32*i:32*i+32 = tile (g0+i) [TxO|TxI]
        for g0 in range(0, T, 4):
            nt = min(4, T - g0)
            ps = psum.tile([P, P], f32, tag="tps")
            nc.tensor.transpose(
                out=ps[0:nt * 2 * CH, :],
                in_=TxC[:, g0:g0 + nt, :].rearrange("p t c -> p (t c)"),
                identity=ident[:])
            for i in range(nt):
                nc.scalar.copy(
                    out=AT[2 * CH:4 * CH, (g0 + i) * P:(g0 + i + 1) * P],
                    in_=ps[i * 2 * CH:(i + 1) * 2 * CH, :])

        # ---- epilogue ----
        out_sb = persist.tile([1, NL], f32)
        CW = 512
        nchunks = (NL + CW - 1) // CW
        for c in range(nchunks):
            lo = c * CW
            w = min(CW, NL - lo)
            pre = psum_pre.tile([P, CW], f32, tag="pre")
            nc.tensor.matmul(out=pre[:, 0:w], lhsT=Wcat[:], rhs=AT[:, lo:lo + w],
                             start=True, stop=True)
            z = work.tile([FILT, CW], f32, tag="z")
            ht = work.tile([FILT, CW], f32, tag="ht")
            # h = relu((1-sigmoid(zpre))*tanh(hpre)) = relu(ht*(0.5-0.5*t))
            # with t = tanh(0.5*zpre + 0.5*b_z), ht = tanh(hpre + b_h)
            nc.scalar.activation(out=z[:, 0:w], in_=pre[0:FILT, 0:w],
                                 func=mybir.ActivationFunctionType.Tanh,
                                 bias=bias_h[0:FILT, :], scale=0.5)
            nc.scalar.activation(out=ht[:, 0:w], in_=pre[FILT:P, 0:w],
                                 func=mybir.ActivationFunctionType.Tanh,
                                 bias=bias_h[FILT:P, :], scale=1.0)
            nc.vector.tensor_scalar(out=z[:, 0:w], in0=z[:, 0:w],
                                    scalar1=-0.5, scalar2=0.5,
                                    op0=mybir.AluOpType.mult,
                                    op1=mybir.AluOpType.add)
            h = work.tile([FILT, CW], f32, tag="h")
            nc.vector.tensor_mul(out=h[:, 0:w], in0=z[:, 0:w], in1=ht[:, 0:w])
            nc.vector.tensor_scalar_max(out=h[:, 0:w], in0=h[:, 0:w], scalar1=0.0)
            ps2 = psum.tile([1, CW], f32, tag="ps2")
            nc.tensor.matmul(out=ps2[:, 0:w], lhsT=lin_w_t[:], rhs=h[:, 0:w],
                             start=True, stop=True)
            nc.vector.tensor_scalar_add(out=out_sb[:, lo:lo + w], in0=ps2[:, 0:w],
                                        scalar1=lin_b_t[0:1, :])
        nc.sync.dma_start(out=out_p[:], in_=out_sb[:])

    nc.compile()
    return nc


# ---------------------------------------------------------------------------
# harness entry point
# ---------------------------------------------------------------------------

_CACHE = {}


def kernel(x, edge_index, edge_weight, w_z, b_z, w_r, b_r, w_h, b_h, lin_w, lin_b):
    """Distributed DCRNN forward on 8 TRN2 NeuronCores.

    Takes full unsharded inputs, returns the full [N, 1] float32 output.
    (w_r/b_r are dead inputs: H0 = 0 makes the reset gate a no-op.)
    """
    from concourse.bass_utils import run_bass_kernel_spmd

    x = np.ascontiguousarray(np.asarray(x, dtype=np.float32))
    cfg, arrays = preprocess(x, np.asarray(edge_index), np.asarray(edge_weight),
                             n_cores=8)
    in_maps = make_in_maps(cfg, arrays, np.asarray(w_z, np.float32),
                           np.asarray(b_z, np.float32),
                           np.asarray(w_h, np.float32),
                           np.asarray(b_h, np.float32),
                           np.asarray(lin_w, np.float32),
                           np.asarray(lin_b, np.float32))
    key = (cfg["N"], cfg["E"], cfg["WA"], cfg["WB"], cfg["KA"], cfg["KB"],
           tuple(tuple(c[:4]) for c in cfg["chunksA"]),
           tuple(tuple(c[:4]) for c in cfg["chunksB"]))
    nc = _CACHE.get(key)
    if nc is None:
        nc = build_kernel(cfg)
        _CACHE[key] = nc
    res = run_bass_kernel_spmd(nc, in_maps, core_ids=list(range(8)))
    return postprocess(cfg, arrays, res.results)


# revision 12
# speedup vs baseline: 1.1095x; 1.1095x over previous
"""Trainium2 Bass kernel for the distributed DCRNN (gnn_message_passing) problem.

Strategy: node-shard across 8 cores (dealt by in-degree rank so all cores share
one compiled grid geometry). All graph indirection is resolved HOST-side by
duplicating INPUT data per edge cell (pure index plumbing — no host arithmetic):

  - xdup[cell]  = x[src(cell)]           (bf16)
  - wdup[cell]  = src's full weight list (bf16, padded to K)

The device re-derives the per-edge scale on-chip (deg = reduce(wdup),
s = 1/deg), forms messages m = xdup * s (bf16, 2x DVE mode), and
segment-reduces per destination. Bulk data moves via regular strided HWDGE
DMAs at full bandwidth.

The A-direction (in-edges per dest) uses in-degree-sorted node order (pi);
the B-direction uses out-degree-sorted order (sigma) for tight tile widths,
and its result TxI is permuted sigma->pi once at node granularity via a
DRAM round-trip + SWDGE dma_gather on the otherwise-idle GpSimd engine,
overlapped under the A-direction stream.
"""

from contextlib import ExitStack

import ml_dtypes
import numpy as np

import concourse.bass as bass
import concourse.bacc as bacc
import concourse.mybir as mybir
import concourse.tile as tile
from concourse.masks import make_identity

P = 128
CH = 16
FILT = 64
CWMAX = 256    # grid columns per streamed chunk
NIMAX = 1024   # max descriptors per dma_gather call (SWDGE ring capacity)


# ---------------------------------------------------------------------------
# host-side preprocessing (index plumbing only; no reference arithmetic)
# ---------------------------------------------------------------------------

def chunk_plan(D, max_width=CWMAX):
    """Split tiles into chunks (contiguous tiles, bounded total width) and
    equal-width runs within each chunk: (t_lo, t_hi, off_lo, off_hi, runs)."""
    T = len(D)
    off = np.concatenate([[0], np.cumsum(D)]).astype(np.int64)
    chunks = []
    t = 0
    while t < T:
        t0 = t
        w = 0
        while t < T and (w + D[t] <= max_width or t == t0):
            w += D[t]
            t += 1
        runs = []
        r = t0
        while r < t:
            r0 = r
            while r < t and D[r] == D[r0]:
                r += 1
            runs.append((r0, r, int(D[r0])))
        chunks.append((t0, t, int(off[t0]), int(off[t]), runs))
    return chunks


def preprocess(x, edge_index, edge_weight, n_cores=8):
    N = x.shape[0]
    E = edge_index.shape[1]
    NPC = N // n_cores
    T = (NPC + P - 1) // P
    NL = P * T
    row = np.ascontiguousarray(edge_index[0]).astype(np.int64)
    col = np.ascontiguousarray(edge_index[1]).astype(np.int64)
    w = np.ascontiguousarray(edge_weight).astype(np.float32)

    cnt_in = np.bincount(col, minlength=N)
    cnt_out = np.bincount(row, minlength=N)

    # deal nodes to cores by global in-degree rank so per-tile degree profiles
    # match across cores (one compiled kernel; minimal tile-width padding)
    g_order = np.argsort(cnt_in, kind="stable")
    cores = np.empty(N, dtype=np.int64)
    cores[g_order] = np.arange(N) % n_cores

    def make_perm(cnt):
        perm = np.full((n_cores, NL), -1, dtype=np.int64)
        pos = np.empty(N, dtype=np.int64)
        for k in range(n_cores):
            nodes = np.where(cores == k)[0]
            order = np.argsort(cnt[nodes], kind="stable")
            perm[k, :NPC] = nodes[order]
            pos[nodes[order]] = np.arange(NPC)
        return perm, pos

    permA, posA = make_perm(cnt_in)   # pi: A-grid dest order, output order
    permB, posB = make_perm(cnt_out)  # sigma: B-grid dest order

    def tile_widths(perm, cnt):
        D = np.zeros(T, dtype=np.int64)
        for k in range(n_cores):
            c = np.where(perm[k] >= 0, cnt[np.maximum(perm[k], 0)], 0)
            D = np.maximum(D, c.reshape(T, P).max(axis=1))
        return np.maximum(D, 1)

    DA = tile_widths(permA, cnt_in)    # A-grid: in-edges per dest (dest = col)
    DB = tile_widths(permB, cnt_out)   # B-grid: out-edges per dest (dest = row)
    offA = np.concatenate([[0], np.cumsum(DA)]).astype(np.int64)
    offB = np.concatenate([[0], np.cumsum(DB)]).astype(np.int64)
    WA, WB = int(offA[-1]), int(offB[-1])

    def kpad(k):
        return 3 * ((int(k) + 2) // 3)
    KA = kpad(cnt_out.max())   # A scale = 1/deg_out(src)
    KB = kpad(cnt_in.max())    # B scale = 1/deg_in(src)

    # padded per-node weight lists (bf16)
    def weight_lists(key, K):
        wp = np.zeros((N, K), dtype=ml_dtypes.bfloat16)
        order = np.argsort(key, kind="stable")
        ks = key[order]
        start = np.concatenate([[0], np.cumsum(np.bincount(ks, minlength=N))])[ks]
        slot = np.arange(E) - start
        wp[ks, slot] = w[order].astype(ml_dtypes.bfloat16)
        return wp
    w_out_pad = weight_lists(row, KA)
    w_in_pad = weight_lists(col, KB)

    xbf = np.asarray(x, dtype=np.float32).astype(ml_dtypes.bfloat16)

    def build_dup(dest, src, pos, off, W, w_pad, K):
        xdup = np.zeros((n_cores, P, W, CH), dtype=ml_dtypes.bfloat16)
        wdup = np.zeros((n_cores, P, W, K), dtype=ml_dtypes.bfloat16)
        k_e = cores[dest]
        j_e = pos[dest]
        t_e, p_e = j_e // P, j_e % P
        order = np.argsort(dest, kind="stable")
        ds = dest[order]
        start = np.concatenate([[0], np.cumsum(np.bincount(ds, minlength=N))])[ds]
        s_e = np.empty(E, dtype=np.int64)
        s_e[order] = np.arange(E) - start
        wcol = off[t_e] + s_e
        xdup[k_e, p_e, wcol] = xbf[src]
        wdup[k_e, p_e, wcol] = w_pad[src]
        return xdup, wdup

    xdupA, wdupA = build_dup(col, row, posA, offA, WA, w_out_pad, KA)
    xdupB, wdupB = build_dup(row, col, posB, offB, WB, w_in_pad, KB)

    # sigma->pi permute index list for the TxI gather: pi cell (p, t) holds
    # node v = permA[k][t*P+p]; its TxIs DRAM row is (posB%P)*T + posB//P.
    # dma_gather linear index i -> output (i%128, i//128), so list order is
    # t-major / p-fast. Wrapped int16 layout: index i at [i%16, i//16],
    # replicated across the 8 Q7 cores (partition groups of 16).
    nip = NIMAX - NIMAX % P
    while NL % nip != 0:
        nip -= P
    ncall = NL // nip
    pad_sig_row = (NPC % P) * T + NPC // P
    permI16 = np.zeros((n_cores, P, NL // 16), dtype=np.int16)
    for k in range(n_cores):
        pk = permA[k]
        sig = np.full(NL, pad_sig_row, dtype=np.int64)
        validA = pk >= 0
        jB = posB[np.maximum(pk, 0)]
        sig[validA] = ((jB % P) * T + jB // P)[validA]
        # sig is indexed by pi j = t*P+p; reorder to i = t*128+p (identical)
        lst = sig.astype(np.int16)                      # [NL] in i order
        wrapped = np.concatenate(
            [lst[c * nip:(c + 1) * nip].reshape(-1, 16).T for c in range(ncall)],
            axis=1)                                     # [16, NL/16]
        permI16[k] = np.tile(wrapped, (8, 1))

    xT = np.zeros((n_cores, CH, NL), dtype=np.float32)
    for k in range(n_cores):
        pk = permA[k]
        valid = pk >= 0
        xg = np.zeros((NL, CH), dtype=np.float32)
        xg[valid] = np.asarray(x, dtype=np.float32)[pk[valid]]
        xT[k] = xg.T

    cfg = dict(
        N=N, E=E, NPC=NPC, T=T, NL=NL, WA=WA, WB=WB, KA=KA, KB=KB,
        n_cores=n_cores, chunksA=chunk_plan(DA), chunksB=chunk_plan(DB),
        nip=nip, ncall=ncall,
    )
    arrays = dict(
        xdupA=xdupA, wdupA=wdupA, xdupB=xdupB, wdupB=wdupB, xT=xT,
        permI16=permI16, permA=permA,
    )
    return cfg, arrays


def make_in_maps(cfg, arrays, w_z, b_z, w_h, b_h, lin_w, lin_b):
    """AT row layout: [x^T (0:16) | zeros (16:32) | TxO^T (32:48) | TxI^T (48:64)].
    Wcat rows match; rows 16:32 are zero (contraction-dim padding is free)."""
    n_cores = cfg["n_cores"]
    w_id0 = np.concatenate([w_z[0, 0, :CH], w_h[0, 0, :CH]], axis=1).astype(np.float32)
    w_id1 = np.concatenate([w_z[1, 0, :CH], w_h[1, 0, :CH]], axis=1).astype(np.float32)
    w_dif = np.concatenate(
        [np.concatenate([w_z[0, 1, :CH], w_h[0, 1, :CH]], axis=1),
         np.concatenate([w_z[1, 1, :CH], w_h[1, 1, :CH]], axis=1)],
        axis=0).astype(np.float32)
    bias = np.concatenate([b_z, b_h]).astype(np.float32).reshape(P, 1)
    in_maps = []
    for k in range(n_cores):
        in_maps.append({
            "xT": np.ascontiguousarray(arrays["xT"][k]),
            "xdupA": np.ascontiguousarray(arrays["xdupA"][k]),
            "wdupA": np.ascontiguousarray(arrays["wdupA"][k]),
            "xdupB": np.ascontiguousarray(arrays["xdupB"][k]),
            "wdupB": np.ascontiguousarray(arrays["wdupB"][k]),
            "permI16": np.ascontiguousarray(arrays["permI16"][k]),
            "w_id0": w_id0, "w_id1": w_id1, "w_dif": w_dif,
            "bias": bias,
            "lin_w": lin_w.astype(np.float32),
            "lin_b": lin_b.astype(np.float32).reshape(1, 1),
        })
    return in_maps


def postprocess(cfg, arrays, results):
    """results[k]['out'] is [1, NL]; scatter back to [N, 1] full output."""
    N, NL = cfg["N"], cfg["NL"]
    out = np.zeros((N, 1), dtype=np.float32)
    for k in range(cfg["n_cores"]):
        o = np.asarray(results[k]["out"]).reshape(NL)
        pk = arrays["permA"][k]
        valid = pk >= 0
        out[pk[valid], 0] = o[valid]
    return out


# ---------------------------------------------------------------------------
# device kernel
# ---------------------------------------------------------------------------

def build_kernel(cfg, debug=False):
    T, NL, WA, WB = cfg["T"], cfg["NL"], cfg["WA"], cfg["WB"]
    KA, KB = cfg["KA"], cfg["KB"]
    nip, ncall = cfg["nip"], cfg["ncall"]
    f32 = mybir.dt.float32
    bf16 = mybir.dt.bfloat16
    i16 = mybir.dt.int16

    nc = bacc.Bacc()

    xT_p = nc.declare_dram_parameter("xT", [CH, NL], f32, isOutput=False)
    xdupA_p = nc.declare_dram_parameter("xdupA", [P, WA, CH], bf16, isOutput=False)
    wdupA_p = nc.declare_dram_parameter("wdupA", [P, WA, KA], bf16, isOutput=False)
    xdupB_p = nc.declare_dram_parameter("xdupB", [P, WB, CH], bf16, isOutput=False)
    wdupB_p = nc.declare_dram_parameter("wdupB", [P, WB, KB], bf16, isOutput=False)
    permI_p = nc.declare_dram_parameter("permI16", [P, NL // 16], i16, isOutput=False)
    w_id0_p = nc.declare_dram_parameter("w_id0", [CH, P], f32, isOutput=False)
    w_id1_p = nc.declare_dram_parameter("w_id1", [CH, P], f32, isOutput=False)
    w_dif_p = nc.declare_dram_parameter("w_dif", [2 * CH, P], f32, isOutput=False)
    bias_p = nc.declare_dram_parameter("bias", [P, 1], f32, isOutput=False)
    lin_w_p = nc.declare_dram_parameter("lin_w", [FILT, 1], f32, isOutput=False)
    lin_b_p = nc.declare_dram_parameter("lin_b", [1, 1], f32, isOutput=False)
    out_p = nc.declare_dram_parameter("out", [1, NL], f32, isOutput=True)

    # TxI bounce: 256B rows (64 f32), channels 0:16 used
    txis_d = nc.dram_tensor("txis", [NL, 64], f32)

    with ExitStack() as ctx:
        tc = ctx.enter_context(tile.TileContext(nc))
        persist = ctx.enter_context(tc.tile_pool(name="persist", bufs=1))
        wpool = ctx.enter_context(tc.tile_pool(name="wpool", bufs=3))
        xpool = ctx.enter_context(tc.tile_pool(name="xpool", bufs=3))
        spool = ctx.enter_context(tc.tile_pool(name="spool", bufs=2))
        mpool = ctx.enter_context(tc.tile_pool(name="mpool", bufs=2))
        work = ctx.enter_context(tc.tile_pool(name="work", bufs=2))
        psum = ctx.enter_context(tc.tile_pool(name="psum", bufs=2, space="PSUM"))
        psum_pre = ctx.enter_context(tc.tile_pool(name="psum_pre", bufs=2, space="PSUM"))

        # ---- persistent tiles & input DMAs ----
        AT = persist.tile([FILT, NL], f32)
        TxC = persist.tile([P, T, 2 * CH], f32)
        TxIs = persist.tile([P, T, CH], f32)
        G2 = persist.tile([P, T, 64], f32)
        permI_t = persist.tile([P, NL // 16], i16)
        Wcat = persist.tile([FILT, P], f32)
        w_id0_t = persist.tile([CH, P], f32)
        w_id1_t = persist.tile([CH, P], f32)
        bias_t = persist.tile([P, 1], f32)
        bias_h = persist.tile([P, 1], f32)
        lin_w_t = persist.tile([FILT, 1], f32)
        lin_b_t = persist.tile([1, 1], f32)
        ident = persist.tile([P, P], f32)

        nc.gpsimd.memset(AT[0:2 * CH, :], 0.0)
        nc.gpsimd.memset(Wcat[0:2 * CH, :], 0.0)
        nc.sync.dma_start(out=AT[0:CH, :], in_=xT_p[:])
        nc.sync.dma_start(out=permI_t[:], in_=permI_p[:])
        nc.sync.dma_start(out=w_id0_t[:], in_=w_id0_p[:])
        nc.sync.dma_start(out=w_id1_t[:], in_=w_id1_p[:])
        nc.sync.dma_start(out=Wcat[2 * CH:4 * CH, :], in_=w_dif_p[:])
        nc.sync.dma_start(out=bias_t[:], in_=bias_p[:])
        nc.sync.dma_start(out=lin_w_t[:], in_=lin_w_p[:])
        nc.sync.dma_start(out=lin_b_t[:], in_=lin_b_p[:])
        make_identity(nc, ident[:])

        nc.vector.tensor_add(out=Wcat[0:CH, :], in0=w_id0_t[:], in1=w_id1_t[:])
        # bias halves: Z-part scaled by 0.5 for the tanh-based sigmoid
        nc.vector.tensor_scalar_mul(out=bias_h[0:FILT, :], in0=bias_t[0:FILT, :],
                                    scalar1=0.5)
        nc.vector.tensor_copy(out=bias_h[FILT:P, :], in_=bias_t[FILT:P, :])

        # ---- streamed message passing ----
        # per chunk: load [P, Wc, K] weights + [P, Wc, CH] features; on-chip
        # deg = reduce(w) (two bf16 2x-mode folds K->K/3, then reduce),
        # s = 1/deg, m = x * s (bf16), segment-reduce into the target.
        # B first: its sigma->pi permute then overlaps the A stream.
        ctx.enter_context(nc.allow_low_precision(
            reason="bf16 edge pipeline; rel tolerance 2e-2 vs bf16 ~4e-3"))

        def stream(xdup_p, wdup_p, Kd, chunks, tx_out, ch0):
            K3 = Kd // 3
            for (t0, t1, o0, o1, runs) in chunks:
                Wc = o1 - o0
                wd = wpool.tile([P, CWMAX, Kd], bf16, tag="wd")
                nc.sync.dma_start(out=wd[:, 0:Wc, :], in_=wdup_p[:, o0:o1, :])
                xd = xpool.tile([P, CWMAX, CH], bf16, tag="xd")
                nc.scalar.dma_start(out=xd[:, 0:Wc, :], in_=xdup_p[:, o0:o1, :])
                fold = mpool.tile([P, CWMAX, K3], bf16, tag="fold")
                nc.vector.tensor_tensor(out=fold[:, 0:Wc, :],
                                        in0=wd[:, 0:Wc, 0:K3],
                                        in1=wd[:, 0:Wc, K3:2 * K3],
                                        op=mybir.AluOpType.add)
                nc.vector.tensor_tensor(out=fold[:, 0:Wc, :],
                                        in0=fold[:, 0:Wc, :],
                                        in1=wd[:, 0:Wc, 2 * K3:Kd],
                                        op=mybir.AluOpType.add)
                s = spool.tile([P, CWMAX], f32, tag="s")
                nc.vector.tensor_reduce(out=s[:, 0:Wc], in_=fold[:, 0:Wc, :],
                                        axis=mybir.AxisListType.X,
                                        op=mybir.AluOpType.add)
                nc.gpsimd.tensor_scalar_max(out=s[:, 0:Wc], in0=s[:, 0:Wc],
                                            scalar1=1e-30)
                sb = spool.tile([P, CWMAX], bf16, tag="sb")
                nc.vector.reciprocal(out=sb[:, 0:Wc], in_=s[:, 0:Wc])
                m = mpool.tile([P, CWMAX, CH], bf16, tag="m")
                nc.vector.tensor_tensor(out=m[:, 0:Wc, :], in0=xd[:, 0:Wc, :],
                                        in1=sb[:, 0:Wc].to_broadcast([P, Wc, CH]),
                                        op=mybir.AluOpType.mult)
                ro = 0
                for (r0, r1, D) in runs:
                    nt = r1 - r0
                    nc.vector.tensor_reduce(
                        out=tx_out[:, r0:r1, ch0:ch0 + CH],
                        in_=m[:, ro:ro + nt * D, :].rearrange(
                            "p (t d) c -> p t c d", t=nt),
                        axis=mybir.AxisListType.X, op=mybir.AluOpType.add)
                    ro += nt * D

        stream(xdupB_p, wdupB_p, KB, cfg["chunksB"], TxIs, 0)

        # sigma->pi permute of TxIs via DRAM round-trip + SWDGE gather
        # (runs on GpSimd, overlapped under the A stream below)
        nc.sync.dma_start(out=txis_d[:, 0:CH], in_=TxIs[:])
        for c in range(ncall):
            nc.gpsimd.dma_gather(
                G2[:, c * (nip // P):(c + 1) * (nip // P), :], txis_d[:],
                permI_t[:, c * (nip // 16):(c + 1) * (nip // 16)],
                nip, nip, 64)

        stream(xdupA_p, wdupA_p, KA, cfg["chunksA"], TxC, 0)

        # emitted AFTER stream A so it does not head-of-line-block the scalar
        # engine's xd chunk loads behind the gather dependency
        nc.scalar.copy(out=TxC[:, :, CH:2 * CH], in_=G2[:, :, 0:CH])

        # ---- transposes into AT rows 32:64 ----
        # 4 tiles per transpose: out rows 32*i:32*i+32 = tile (g0+i) [TxO|TxI]
        for g0 in range(0, T, 4):
            nt = min(4, T - g0)
            ps = psum.tile([P, P], f32, tag="tps")
            nc.tensor.transpose(
                out=ps[0:nt * 2 * CH, :],
                in_=TxC[:, g0:g0 + nt, :].rearrange("p t c -> p (t c)"),
                identity=ident[:])
            for i in range(nt):
                nc.scalar.copy(
                    out=AT[2 * CH:4 * CH, (g0 + i) * P:(g0 + i + 1) * P],
                    in_=ps[i * 2 * CH:(i + 1) * 2 * CH, :])

        # ---- epilogue ----
        out_sb = persist.tile([1, NL], f32)
        CW = 512
        nchunks = (NL + CW - 1) // CW
        for c in range(nchunks):
            lo = c * CW
            w = min(CW, NL - lo)
            pre = psum_pre.tile([P, CW], f32, tag="pre")
            nc.tensor.matmul(out=pre[:, 0:w], lhsT=Wcat[:], rhs=AT[:, lo:lo + w],
                             start=True, stop=True)
            z = work.tile([FILT, CW], f32, tag="z")
            ht = work.tile([FILT, CW], f32, tag="ht")
            # h = relu((1-sigmoid(zpre))*tanh(hpre)) = relu(ht*(0.5-0.5*t))
            # with t = tanh(0.5*zpre + 0.5*b_z), ht = tanh(hpre + b_h)
            nc.scalar.activation(out=z[:, 0:w], in_=pre[0:FILT, 0:w],
                                 func=mybir.ActivationFunctionType.Tanh,
                                 bias=bias_h[0:FILT, :], scale=0.5)
            nc.scalar.activation(out=ht[:, 0:w], in_=pre[FILT:P, 0:w],
                                 func=mybir.ActivationFunctionType.Tanh,
                                 bias=bias_h[FILT:P, :], scale=1.0)
            nc.vector.tensor_scalar(out=z[:, 0:w], in0=z[:, 0:w],
                                    scalar1=-0.5, scalar2=0.5,
                                    op0=mybir.AluOpType.mult,
                                    op1=mybir.AluOpType.add)
            h = work.tile([FILT, CW], f32, tag="h")
            nc.vector.tensor_mul(out=h[:, 0:w], in0=z[:, 0:w], in1=ht[:, 0:w])
            nc.vector.tensor_scalar_max(out=h[:, 0:w], in0=h[:, 0:w], scalar1=0.0)
            ps2 = psum.tile([1, CW], f32, tag="ps2")
            nc.tensor.matmul(out=ps2[:, 0:w], lhsT=lin_w_t[:], rhs=h[:, 0:w],
                             start=True, stop=True)
            nc.vector.tensor_scalar_add(out=out_sb[:, lo:lo + w], in0=ps2[:, 0:w],
                                        scalar1=lin_b_t[0:1, :])
        nc.sync.dma_start(out=out_p[:], in_=out_sb[:])

    nc.compile()
    return nc


# ---------------------------------------------------------------------------
# harness entry point
# ---------------------------------------------------------------------------

_CACHE = {}


def kernel(x, edge_index, edge_weight, w_z, b_z, w_r, b_r, w_h, b_h, lin_w, lin_b):
    """Distributed DCRNN forward on 8 TRN2 NeuronCores.

    Takes full unsharded inputs, returns the full [N, 1] float32 output.
    (w_r/b_r are dead inputs: H0 = 0 makes the reset gate a no-op.)
    """
    from concourse.bass_utils import run_bass_kernel_spmd

    x = np.ascontiguousarray(np.asarray(x, dtype=np.float32))
    cfg, arrays = preprocess(x, np.asarray(edge_index), np.asarray(edge_weight),
                             n_cores=8)
    in_maps = make_in_maps(cfg, arrays, np.asarray(w_z, np.float32),
                           np.asarray(b_z, np.float32),
                           np.asarray(w_h, np.float32),
                           np.asarray(b_h, np.float32),
                           np.asarray(lin_w, np.float32),
                           np.asarray(lin_b, np.float32))
    key = (cfg["N"], cfg["E"], cfg["WA"], cfg["WB"], cfg["KA"], cfg["KB"],
           tuple(tuple(c[:4]) for c in cfg["chunksA"]),
           tuple(tuple(c[:4]) for c in cfg["chunksB"]))
    nc = _CACHE.get(key)
    if nc is None:
        nc = build_kernel(cfg)
        _CACHE[key] = nc
    res = run_bass_kernel_spmd(nc, in_maps, core_ids=list(range(8)))
    return postprocess(cfg, arrays, res.results)


# revision 19
# speedup vs baseline: 1.2166x; 1.0965x over previous
"""Trainium2 Bass kernel for the distributed DCRNN (gnn_message_passing) problem.

Strategy: node-shard across 8 cores (dealt by in-degree rank so all cores share
one compiled grid geometry). All graph indirection is resolved HOST-side by
duplicating INPUT data per edge cell (pure index plumbing — no host arithmetic):

  - xdup[cell]  = x[src(cell)]           (bf16)
  - wdup[cell]  = src's full weight list (bf16, padded to K)

The device re-derives the per-edge scale on-chip (deg = reduce(wdup),
s = 1/deg), forms messages m = xdup * s (bf16, 2x DVE mode), and
segment-reduces per destination. Bulk data moves via regular strided HWDGE
DMAs at full bandwidth.

The A-direction (in-edges per dest) uses in-degree-sorted node order (pi);
the B-direction uses out-degree-sorted order (sigma) for tight tile widths,
and its result TxI is permuted sigma->pi once at node granularity via a
DRAM round-trip + SWDGE dma_gather on the otherwise-idle GpSimd engine,
overlapped under the A-direction stream.
"""

from contextlib import ExitStack

import ml_dtypes
import numpy as np

import concourse.bass as bass
import concourse.bacc as bacc
import concourse.mybir as mybir
import concourse.tile as tile
from concourse.masks import make_identity

P = 128
CH = 16
FILT = 64
CWMAX = 256    # grid columns per streamed chunk
NIMAX = 1024   # max descriptors per dma_gather call (SWDGE ring capacity)


# ---------------------------------------------------------------------------
# host-side preprocessing (index plumbing only; no reference arithmetic)
# ---------------------------------------------------------------------------

def chunk_plan(D, max_width=CWMAX):
    """Split tiles into chunks (contiguous tiles, bounded total width) and
    equal-width runs within each chunk: (t_lo, t_hi, off_lo, off_hi, runs)."""
    T = len(D)
    off = np.concatenate([[0], np.cumsum(D)]).astype(np.int64)
    chunks = []
    t = 0
    while t < T:
        t0 = t
        w = 0
        while t < T and (w + D[t] <= max_width or t == t0):
            w += D[t]
            t += 1
        runs = []
        r = t0
        while r < t:
            r0 = r
            while r < t and D[r] == D[r0]:
                r += 1
            runs.append((r0, r, int(D[r0])))
        chunks.append((t0, t, int(off[t0]), int(off[t]), runs))
    return chunks


def preprocess(x, edge_index, edge_weight, n_cores=8):
    N = x.shape[0]
    E = edge_index.shape[1]
    NPC = N // n_cores
    T = (NPC + P - 1) // P
    NL = P * T
    row = np.ascontiguousarray(edge_index[0]).astype(np.int64)
    col = np.ascontiguousarray(edge_index[1]).astype(np.int64)
    w = np.ascontiguousarray(edge_weight).astype(np.float32)

    cnt_in = np.bincount(col, minlength=N)
    cnt_out = np.bincount(row, minlength=N)

    # deal nodes to cores by global in-degree rank so per-tile degree profiles
    # match across cores (one compiled kernel; minimal tile-width padding)
    g_order = np.argsort(cnt_in, kind="stable")
    cores = np.empty(N, dtype=np.int64)
    cores[g_order] = np.arange(N) % n_cores

    def make_perm(cnt):
        perm = np.full((n_cores, NL), -1, dtype=np.int64)
        pos = np.empty(N, dtype=np.int64)
        for k in range(n_cores):
            nodes = np.where(cores == k)[0]
            order = np.argsort(cnt[nodes], kind="stable")
            perm[k, :NPC] = nodes[order]
            pos[nodes[order]] = np.arange(NPC)
        return perm, pos

    permA, posA = make_perm(cnt_in)   # pi: A-grid dest order, output order
    permB, posB = make_perm(cnt_out)  # sigma: B-grid dest order

    def tile_widths(perm, cnt):
        D = np.zeros(T, dtype=np.int64)
        for k in range(n_cores):
            c = np.where(perm[k] >= 0, cnt[np.maximum(perm[k], 0)], 0)
            D = np.maximum(D, c.reshape(T, P).max(axis=1))
        return np.maximum(D, 1)

    DA = tile_widths(permA, cnt_in)    # A-grid: in-edges per dest (dest = col)
    DB = tile_widths(permB, cnt_out)   # B-grid: out-edges per dest (dest = row)
    offA = np.concatenate([[0], np.cumsum(DA)]).astype(np.int64)
    offB = np.concatenate([[0], np.cumsum(DB)]).astype(np.int64)
    WA, WB = int(offA[-1]), int(offB[-1])

    def kpad(k):
        return 9 * ((int(k) + 8) // 9)
    KA = kpad(cnt_out.max())   # A scale = 1/deg_out(src)
    KB = kpad(cnt_in.max())    # B scale = 1/deg_in(src)

    # padded per-node weight lists (bf16)
    def weight_lists(key, K):
        wp = np.zeros((N, K), dtype=ml_dtypes.bfloat16)
        order = np.argsort(key, kind="stable")
        ks = key[order]
        start = np.concatenate([[0], np.cumsum(np.bincount(ks, minlength=N))])[ks]
        slot = np.arange(E) - start
        wp[ks, slot] = w[order].astype(ml_dtypes.bfloat16)
        return wp
    w_out_pad = weight_lists(row, KA)
    w_in_pad = weight_lists(col, KB)

    xbf = np.asarray(x, dtype=np.float32).astype(ml_dtypes.bfloat16)

    def build_dup(dest, src, pos, off, W, w_pad, K):
        xdup = np.zeros((n_cores, P, W, CH), dtype=ml_dtypes.bfloat16)
        wdup = np.zeros((n_cores, P, W, K), dtype=ml_dtypes.bfloat16)
        k_e = cores[dest]
        j_e = pos[dest]
        t_e, p_e = j_e // P, j_e % P
        order = np.argsort(dest, kind="stable")
        ds = dest[order]
        start = np.concatenate([[0], np.cumsum(np.bincount(ds, minlength=N))])[ds]
        s_e = np.empty(E, dtype=np.int64)
        s_e[order] = np.arange(E) - start
        wcol = off[t_e] + s_e
        xdup[k_e, p_e, wcol] = xbf[src]
        wdup[k_e, p_e, wcol] = w_pad[src]
        return xdup, wdup

    xdupA, wdupA = build_dup(col, row, posA, offA, WA, w_out_pad, KA)
    xdupB, wdupB = build_dup(row, col, posB, offB, WB, w_in_pad, KB)

    # sigma->pi permute index list for the TxI gather: pi cell (p, t) holds
    # node v = permA[k][t*P+p]; its TxIs DRAM row is (posB%P)*T + posB//P.
    # dma_gather linear index i -> output (i%128, i//128), so list order is
    # t-major / p-fast. Wrapped int16 layout: index i at [i%16, i//16],
    # replicated across the 8 Q7 cores (partition groups of 16).
    nip = NIMAX - NIMAX % P
    while NL % nip != 0:
        nip -= P
    ncall = NL // nip
    pad_sig_row = (NPC % P) * T + NPC // P
    permI16 = np.zeros((n_cores, P, NL // 16), dtype=np.int16)
    for k in range(n_cores):
        pk = permA[k]
        sig = np.full(NL, pad_sig_row, dtype=np.int64)
        validA = pk >= 0
        jB = posB[np.maximum(pk, 0)]
        sig[validA] = ((jB % P) * T + jB // P)[validA]
        # sig is indexed by pi j = t*P+p; reorder to i = t*128+p (identical)
        lst = sig.astype(np.int16)                      # [NL] in i order
        wrapped = np.concatenate(
            [lst[c * nip:(c + 1) * nip].reshape(-1, 16).T for c in range(ncall)],
            axis=1)                                     # [16, NL/16]
        permI16[k] = np.tile(wrapped, (8, 1))

    xT = np.zeros((n_cores, CH, NL), dtype=np.float32)
    for k in range(n_cores):
        pk = permA[k]
        valid = pk >= 0
        xg = np.zeros((NL, CH), dtype=np.float32)
        xg[valid] = np.asarray(x, dtype=np.float32)[pk[valid]]
        xT[k] = xg.T

    cfg = dict(
        N=N, E=E, NPC=NPC, T=T, NL=NL, WA=WA, WB=WB, KA=KA, KB=KB,
        n_cores=n_cores, chunksA=chunk_plan(DA), chunksB=chunk_plan(DB),
        nip=nip, ncall=ncall,
    )
    arrays = dict(
        xdupA=xdupA, wdupA=wdupA, xdupB=xdupB, wdupB=wdupB, xT=xT,
        permI16=permI16, permA=permA,
    )
    return cfg, arrays


def make_in_maps(cfg, arrays, w_z, b_z, w_h, b_h, lin_w, lin_b):
    """AT row layout: [x^T (0:16) | zeros (16:32) | TxO^T (32:48) | TxI^T (48:64)].
    Wcat rows match; rows 16:32 are zero (contraction-dim padding is free)."""
    n_cores = cfg["n_cores"]
    w_id0 = np.concatenate([w_z[0, 0, :CH], w_h[0, 0, :CH]], axis=1).astype(np.float32)
    w_id1 = np.concatenate([w_z[1, 0, :CH], w_h[1, 0, :CH]], axis=1).astype(np.float32)
    w_dif = np.concatenate(
        [np.concatenate([w_z[0, 1, :CH], w_h[0, 1, :CH]], axis=1),
         np.concatenate([w_z[1, 1, :CH], w_h[1, 1, :CH]], axis=1)],
        axis=0).astype(np.float32)
    bias = np.concatenate([b_z, b_h]).astype(np.float32).reshape(P, 1)
    in_maps = []
    for k in range(n_cores):
        in_maps.append({
            "xT": np.ascontiguousarray(arrays["xT"][k]),
            "xdupA": np.ascontiguousarray(arrays["xdupA"][k]),
            "wdupA": np.ascontiguousarray(arrays["wdupA"][k]),
            "xdupB": np.ascontiguousarray(arrays["xdupB"][k]),
            "wdupB": np.ascontiguousarray(arrays["wdupB"][k]),
            "permI16": np.ascontiguousarray(arrays["permI16"][k]),
            "w_id0": w_id0, "w_id1": w_id1, "w_dif": w_dif,
            "bias": bias,
            "lin_w": lin_w.astype(np.float32),
            "lin_b": lin_b.astype(np.float32).reshape(1, 1),
        })
    return in_maps


def postprocess(cfg, arrays, results):
    """results[k]['out'] is [1, NL]; scatter back to [N, 1] full output."""
    N, NL = cfg["N"], cfg["NL"]
    out = np.zeros((N, 1), dtype=np.float32)
    for k in range(cfg["n_cores"]):
        o = np.asarray(results[k]["out"]).reshape(NL)
        pk = arrays["permA"][k]
        valid = pk >= 0
        out[pk[valid], 0] = o[valid]
    return out


# ---------------------------------------------------------------------------
# device kernel
# ---------------------------------------------------------------------------

def build_kernel(cfg, debug=False):
    T, NL, WA, WB = cfg["T"], cfg["NL"], cfg["WA"], cfg["WB"]
    KA, KB = cfg["KA"], cfg["KB"]
    nip, ncall = cfg["nip"], cfg["ncall"]
    f32 = mybir.dt.float32
    bf16 = mybir.dt.bfloat16
    i16 = mybir.dt.int16

    nc = bacc.Bacc()

    xT_p = nc.declare_dram_parameter("xT", [CH, NL], f32, isOutput=False)
    xdupA_p = nc.declare_dram_parameter("xdupA", [P, WA, CH], bf16, isOutput=False)
    wdupA_p = nc.declare_dram_parameter("wdupA", [P, WA, KA], bf16, isOutput=False)
    xdupB_p = nc.declare_dram_parameter("xdupB", [P, WB, CH], bf16, isOutput=False)
    wdupB_p = nc.declare_dram_parameter("wdupB", [P, WB, KB], bf16, isOutput=False)
    permI_p = nc.declare_dram_parameter("permI16", [P, NL // 16], i16, isOutput=False)
    w_id0_p = nc.declare_dram_parameter("w_id0", [CH, P], f32, isOutput=False)
    w_id1_p = nc.declare_dram_parameter("w_id1", [CH, P], f32, isOutput=False)
    w_dif_p = nc.declare_dram_parameter("w_dif", [2 * CH, P], f32, isOutput=False)
    bias_p = nc.declare_dram_parameter("bias", [P, 1], f32, isOutput=False)
    lin_w_p = nc.declare_dram_parameter("lin_w", [FILT, 1], f32, isOutput=False)
    lin_b_p = nc.declare_dram_parameter("lin_b", [1, 1], f32, isOutput=False)
    out_p = nc.declare_dram_parameter("out", [1, NL], f32, isOutput=True)

    # TxI bounce: 256B rows (64 f32), channels 0:16 used
    txis_d = nc.dram_tensor("txis", [NL, 64], f32)

    with ExitStack() as ctx:
        tc = ctx.enter_context(tile.TileContext(nc))
        persist = ctx.enter_context(tc.tile_pool(name="persist", bufs=1))
        wpool = ctx.enter_context(tc.tile_pool(name="wpool", bufs=3))
        xpool = ctx.enter_context(tc.tile_pool(name="xpool", bufs=3))
        spool = ctx.enter_context(tc.tile_pool(name="spool", bufs=2))
        mpool = ctx.enter_context(tc.tile_pool(name="mpool", bufs=2))
        work = ctx.enter_context(tc.tile_pool(name="work", bufs=2))
        psum = ctx.enter_context(tc.tile_pool(name="psum", bufs=2, space="PSUM"))
        psum_pre = ctx.enter_context(tc.tile_pool(name="psum_pre", bufs=2, space="PSUM"))

        # ---- persistent tiles & input DMAs ----
        AT = persist.tile([FILT, NL], f32)
        TxC = persist.tile([P, T, 2 * CH], f32)
        TxIs = persist.tile([P, T, CH], f32)
        G2 = persist.tile([P, T, 64], f32)
        permI_t = persist.tile([P, NL // 16], i16)
        Wcat = persist.tile([FILT, P], f32)
        w_id0_t = persist.tile([CH, P], f32)
        w_id1_t = persist.tile([CH, P], f32)
        bias_t = persist.tile([P, 1], f32)
        bias_h = persist.tile([P, 1], f32)
        lin_w_t = persist.tile([FILT, 1], f32)
        lin_b_t = persist.tile([1, 1], f32)
        ident = persist.tile([P, P], f32)

        nc.gpsimd.memset(AT[0:2 * CH, :], 0.0)
        nc.gpsimd.memset(Wcat[0:2 * CH, :], 0.0)
        nc.sync.dma_start(out=AT[0:CH, :], in_=xT_p[:])
        nc.sync.dma_start(out=permI_t[:], in_=permI_p[:])
        nc.sync.dma_start(out=w_id0_t[:], in_=w_id0_p[:])
        nc.sync.dma_start(out=w_id1_t[:], in_=w_id1_p[:])
        nc.sync.dma_start(out=Wcat[2 * CH:4 * CH, :], in_=w_dif_p[:])
        nc.sync.dma_start(out=bias_t[:], in_=bias_p[:])
        nc.sync.dma_start(out=lin_w_t[:], in_=lin_w_p[:])
        nc.sync.dma_start(out=lin_b_t[:], in_=lin_b_p[:])
        make_identity(nc, ident[:])

        nc.vector.tensor_add(out=Wcat[0:CH, :], in0=w_id0_t[:], in1=w_id1_t[:])
        lin_w_bf = persist.tile([FILT, 1], bf16)
        nc.scalar.copy(out=lin_w_bf[:], in_=lin_w_t[:])
        # bias halves: Z-part scaled by 0.5 for the tanh-based sigmoid
        nc.vector.tensor_scalar_mul(out=bias_h[0:FILT, :], in0=bias_t[0:FILT, :],
                                    scalar1=0.5)
        nc.vector.tensor_copy(out=bias_h[FILT:P, :], in_=bias_t[FILT:P, :])

        # ---- streamed message passing ----
        # per chunk: load [P, Wc, K] weights + [P, Wc, CH] features; on-chip
        # deg = reduce(w) (two bf16 2x-mode folds K->K/3, then reduce),
        # s = 1/deg, m = x * s (bf16), segment-reduce into the target.
        # B first: its sigma->pi permute then overlaps the A stream.
        ctx.enter_context(nc.allow_low_precision(
            reason="bf16 edge pipeline; rel tolerance 2e-2 vs bf16 ~4e-3"))

        def stream(xdup_p, wdup_p, Kd, chunks, tx_out, ch0):
            K3 = Kd // 3
            for (t0, t1, o0, o1, runs) in chunks:
                Wc = o1 - o0
                wd = wpool.tile([P, CWMAX, Kd], bf16, tag="wd")
                nc.sync.dma_start(out=wd[:, 0:Wc, :], in_=wdup_p[:, o0:o1, :])
                xd = xpool.tile([P, CWMAX, CH], bf16, tag="xd")
                nc.scalar.dma_start(out=xd[:, 0:Wc, :], in_=xdup_p[:, o0:o1, :])
                fold = mpool.tile([P, CWMAX, K3], bf16, tag="fold")
                nc.vector.tensor_tensor(out=fold[:, 0:Wc, :],
                                        in0=wd[:, 0:Wc, 0:K3],
                                        in1=wd[:, 0:Wc, K3:2 * K3],
                                        op=mybir.AluOpType.add)
                nc.vector.tensor_tensor(out=fold[:, 0:Wc, :],
                                        in0=fold[:, 0:Wc, :],
                                        in1=wd[:, 0:Wc, 2 * K3:Kd],
                                        op=mybir.AluOpType.add)
                K9 = K3 // 3
                fold2 = mpool.tile([P, CWMAX, K9], bf16, tag="fold2")
                nc.vector.tensor_tensor(out=fold2[:, 0:Wc, :],
                                        in0=fold[:, 0:Wc, 0:K9],
                                        in1=fold[:, 0:Wc, K9:2 * K9],
                                        op=mybir.AluOpType.add)
                nc.vector.tensor_tensor(out=fold2[:, 0:Wc, :],
                                        in0=fold2[:, 0:Wc, :],
                                        in1=fold[:, 0:Wc, 2 * K9:K3],
                                        op=mybir.AluOpType.add)
                s = spool.tile([P, CWMAX], f32, tag="s")
                nc.vector.tensor_reduce(out=s[:, 0:Wc], in_=fold2[:, 0:Wc, :],
                                        axis=mybir.AxisListType.X,
                                        op=mybir.AluOpType.add)
                nc.vector.tensor_scalar_max(out=s[:, 0:Wc], in0=s[:, 0:Wc],
                                            scalar1=1e-30)
                sb = spool.tile([P, CWMAX], bf16, tag="sb")
                nc.vector.reciprocal(out=sb[:, 0:Wc], in_=s[:, 0:Wc])
                m = mpool.tile([P, CWMAX, CH], bf16, tag="m")
                nc.vector.tensor_tensor(out=m[:, 0:Wc, :], in0=xd[:, 0:Wc, :],
                                        in1=sb[:, 0:Wc].to_broadcast([P, Wc, CH]),
                                        op=mybir.AluOpType.mult)
                ro = 0
                for (r0, r1, D) in runs:
                    nt = r1 - r0
                    nc.vector.tensor_reduce(
                        out=tx_out[:, r0:r1, ch0:ch0 + CH],
                        in_=m[:, ro:ro + nt * D, :].rearrange(
                            "p (t d) c -> p t c d", t=nt),
                        axis=mybir.AxisListType.X, op=mybir.AluOpType.add)
                    ro += nt * D

        stream(xdupB_p, wdupB_p, KB, cfg["chunksB"], TxIs, 0)

        # sigma->pi permute of TxIs via DRAM round-trip + SWDGE gather
        # (runs on GpSimd, overlapped under the A stream below)
        nc.sync.dma_start(out=txis_d[:, 0:CH], in_=TxIs[:])
        for c in range(ncall):
            nc.gpsimd.dma_gather(
                G2[:, c * (nip // P):(c + 1) * (nip // P), :], txis_d[:],
                permI_t[:, c * (nip // 16):(c + 1) * (nip // 16)],
                nip, nip, 64)

        stream(xdupA_p, wdupA_p, KA, cfg["chunksA"], TxC, 0)

        # emitted AFTER stream A so they do not head-of-line-block the scalar
        # engine's xd chunk loads; split per gather so transposes pipeline
        for c in range(ncall):
            t0c, t1c = c * (nip // P), (c + 1) * (nip // P)
            nc.scalar.copy(out=TxC[:, t0c:t1c, CH:2 * CH],
                           in_=G2[:, t0c:t1c, 0:CH])

        # ---- transposes into AT rows 32:64 ----
        # 4 tiles per transpose: out rows 32*i:32*i+32 = tile (g0+i) [TxO|TxI]
        for g0 in range(0, T, 4):
            nt = min(4, T - g0)
            ps = psum.tile([P, P], f32, tag="tps")
            nc.tensor.transpose(
                out=ps[0:nt * 2 * CH, :],
                in_=TxC[:, g0:g0 + nt, :].rearrange("p t c -> p (t c)"),
                identity=ident[:])
            for i in range(nt):
                nc.scalar.copy(
                    out=AT[2 * CH:4 * CH, (g0 + i) * P:(g0 + i + 1) * P],
                    in_=ps[i * 2 * CH:(i + 1) * 2 * CH, :])

        # ---- epilogue ----
        out_sb = persist.tile([1, NL], f32)
        CW = 512
        nchunks = (NL + CW - 1) // CW
        for c in range(nchunks):
            lo = c * CW
            w = min(CW, NL - lo)
            pre = psum_pre.tile([P, CW], f32, tag="pre")
            nc.tensor.matmul(out=pre[:, 0:w], lhsT=Wcat[:], rhs=AT[:, lo:lo + w],
                             start=True, stop=True)
            z = work.tile([FILT, CW], f32, tag="z")
            ht = work.tile([FILT, CW], f32, tag="ht")
            # h = relu((1-sigmoid(zpre))*tanh(hpre)) = relu(ht*(0.5-0.5*t))
            # with t = tanh(0.5*zpre + 0.5*b_z), ht = tanh(hpre + b_h)
            nc.scalar.activation(out=z[:, 0:w], in_=pre[0:FILT, 0:w],
                                 func=mybir.ActivationFunctionType.Tanh,
                                 bias=bias_h[0:FILT, :], scale=0.5)
            nc.scalar.activation(out=ht[:, 0:w], in_=pre[FILT:P, 0:w],
                                 func=mybir.ActivationFunctionType.Tanh,
                                 bias=bias_h[FILT:P, :], scale=1.0)
            nc.vector.tensor_scalar(out=z[:, 0:w], in0=z[:, 0:w],
                                    scalar1=-0.5, scalar2=0.5,
                                    op0=mybir.AluOpType.mult,
                                    op1=mybir.AluOpType.add)
            h = work.tile([FILT, CW], f32, tag="h")
            nc.vector.tensor_mul(out=h[:, 0:w], in0=z[:, 0:w], in1=ht[:, 0:w])
            nc.vector.tensor_scalar_max(out=h[:, 0:w], in0=h[:, 0:w], scalar1=0.0)
            ps2 = psum.tile([1, CW], f32, tag="ps2")
            nc.tensor.matmul(out=ps2[:, 0:w], lhsT=lin_w_t[:], rhs=h[:, 0:w],
                             start=True, stop=True)
            nc.vector.tensor_scalar_add(out=out_sb[:, lo:lo + w], in0=ps2[:, 0:w],
                                        scalar1=lin_b_t[0:1, :])
        nc.sync.dma_start(out=out_p[:], in_=out_sb[:])

    nc.compile()
    return nc


# ---------------------------------------------------------------------------
# harness entry point
# ---------------------------------------------------------------------------

_CACHE = {}


def kernel(x, edge_index, edge_weight, w_z, b_z, w_r, b_r, w_h, b_h, lin_w, lin_b):
    """Distributed DCRNN forward on 8 TRN2 NeuronCores.

    Takes full unsharded inputs, returns the full [N, 1] float32 output.
    (w_r/b_r are dead inputs: H0 = 0 makes the reset gate a no-op.)
    """
    from concourse.bass_utils import run_bass_kernel_spmd

    x = np.ascontiguousarray(np.asarray(x, dtype=np.float32))
    cfg, arrays = preprocess(x, np.asarray(edge_index), np.asarray(edge_weight),
                             n_cores=8)
    in_maps = make_in_maps(cfg, arrays, np.asarray(w_z, np.float32),
                           np.asarray(b_z, np.float32),
                           np.asarray(w_h, np.float32),
                           np.asarray(b_h, np.float32),
                           np.asarray(lin_w, np.float32),
                           np.asarray(lin_b, np.float32))
    key = (cfg["N"], cfg["E"], cfg["WA"], cfg["WB"], cfg["KA"], cfg["KB"],
           tuple(tuple(c[:4]) for c in cfg["chunksA"]),
           tuple(tuple(c[:4]) for c in cfg["chunksB"]))
    nc = _CACHE.get(key)
    if nc is None:
        nc = build_kernel(cfg)
        _CACHE[key] = nc
    res = run_bass_kernel_spmd(nc, in_maps, core_ids=list(range(8)))
    return postprocess(cfg, arrays, res.results)


# revision 29
# speedup vs baseline: 1.4265x; 1.1725x over previous
"""Trainium2 Bass kernel for the distributed DCRNN (gnn_message_passing) problem.

Strategy: node-shard across 8 cores (dealt by in-degree rank so all cores share
one compiled grid geometry). All graph indirection is resolved HOST-side by
duplicating INPUT data per edge cell (pure index plumbing — no host arithmetic):

  - xdup[cell]  = x[src(cell)]           (bf16)
  - wdup[cell]  = src's full weight list (bf16, padded to K)

The device re-derives the per-edge scale on-chip (deg = reduce(wdup),
s = 1/deg), forms messages m = xdup * s (bf16, 2x DVE mode), and
segment-reduces per destination. Bulk data moves via regular strided HWDGE
DMAs at full bandwidth.

The A-direction (in-edges per dest) uses in-degree-sorted node order (pi);
the B-direction uses out-degree-sorted order (sigma) for tight tile widths,
and its result TxI is permuted sigma->pi once at node granularity via a
DRAM round-trip + SWDGE dma_gather on the otherwise-idle GpSimd engine,
overlapped under the A-direction stream.
"""

from contextlib import ExitStack

import ml_dtypes
import numpy as np

import concourse.bass as bass
import concourse.bacc as bacc
import concourse.mybir as mybir
import concourse.tile as tile
from concourse.masks import make_identity

P = 128
CH = 16
FILT = 64
CWMAX = 256    # grid columns per streamed chunk
NIMAX = 1024   # max descriptors per dma_gather call (SWDGE ring capacity)


# ---------------------------------------------------------------------------
# host-side preprocessing (index plumbing only; no reference arithmetic)
# ---------------------------------------------------------------------------

def chunk_plan(D, max_width=CWMAX, first_width=64):
    """Split tiles into chunks (contiguous tiles, bounded total width) and
    equal-width runs within each chunk: (t_lo, t_hi, off_lo, off_hi, runs).
    The first chunk is kept small so its DMA lands early (short ramp)."""
    T = len(D)
    off = np.concatenate([[0], np.cumsum(D)]).astype(np.int64)
    chunks = []
    t = 0
    while t < T:
        t0 = t
        w = 0
        mw = first_width if t0 == 0 else max_width
        while t < T and (w + D[t] <= mw or t == t0):
            w += D[t]
            t += 1
        runs = []
        r = t0
        while r < t:
            r0 = r
            while r < t and D[r] == D[r0]:
                r += 1
            runs.append((r0, r, int(D[r0])))
        chunks.append((t0, t, int(off[t0]), int(off[t]), runs))
    return chunks


def preprocess(x, edge_index, edge_weight, n_cores=8):
    N = x.shape[0]
    E = edge_index.shape[1]
    NPC = N // n_cores
    T = (NPC + P - 1) // P
    NL = P * T
    row = np.ascontiguousarray(edge_index[0]).astype(np.int64)
    col = np.ascontiguousarray(edge_index[1]).astype(np.int64)
    w = np.ascontiguousarray(edge_weight).astype(np.float32)

    cnt_in = np.bincount(col, minlength=N)
    cnt_out = np.bincount(row, minlength=N)

    # deal nodes to cores by global in-degree rank so per-tile degree profiles
    # match across cores (one compiled kernel; minimal tile-width padding)
    g_order = np.argsort(cnt_in, kind="stable")
    cores = np.empty(N, dtype=np.int64)
    cores[g_order] = np.arange(N) % n_cores

    def make_perm(cnt):
        perm = np.full((n_cores, NL), -1, dtype=np.int64)
        pos = np.empty(N, dtype=np.int64)
        for k in range(n_cores):
            nodes = np.where(cores == k)[0]
            order = np.argsort(cnt[nodes], kind="stable")
            perm[k, :NPC] = nodes[order]
            pos[nodes[order]] = np.arange(NPC)
        return perm, pos

    permA, posA = make_perm(cnt_in)   # pi: A-grid dest order, output order
    permB, posB = make_perm(cnt_out)  # sigma: B-grid dest order

    def tile_widths(perm, cnt):
        D = np.zeros(T, dtype=np.int64)
        for k in range(n_cores):
            c = np.where(perm[k] >= 0, cnt[np.maximum(perm[k], 0)], 0)
            D = np.maximum(D, c.reshape(T, P).max(axis=1))
        return np.maximum(D, 1)

    DA = tile_widths(permA, cnt_in)    # A-grid: in-edges per dest (dest = col)
    DB = tile_widths(permB, cnt_out)   # B-grid: out-edges per dest (dest = row)
    offA = np.concatenate([[0], np.cumsum(DA)]).astype(np.int64)
    offB = np.concatenate([[0], np.cumsum(DB)]).astype(np.int64)
    WA, WB = int(offA[-1]), int(offB[-1])

    def kpad(k):
        return 9 * ((int(k) + 8) // 9)
    KA = kpad(cnt_out.max())   # A scale = 1/deg_out(src)
    KB = kpad(cnt_in.max())    # B scale = 1/deg_in(src)

    # padded per-node weight lists (bf16)
    def weight_lists(key, K):
        wp = np.zeros((N, K), dtype=ml_dtypes.bfloat16)
        order = np.argsort(key, kind="stable")
        ks = key[order]
        start = np.concatenate([[0], np.cumsum(np.bincount(ks, minlength=N))])[ks]
        slot = np.arange(E) - start
        wp[ks, slot] = w[order].astype(ml_dtypes.bfloat16)
        return wp
    w_out_pad = weight_lists(row, KA)
    w_in_pad = weight_lists(col, KB)

    xbf = np.asarray(x, dtype=np.float32).astype(ml_dtypes.bfloat16)

    def build_dup(dest, src, pos, off, W, w_pad, K):
        xdup = np.zeros((n_cores, P, W, CH), dtype=ml_dtypes.bfloat16)
        wdup = np.zeros((n_cores, P, W, K), dtype=ml_dtypes.bfloat16)
        k_e = cores[dest]
        j_e = pos[dest]
        t_e, p_e = j_e // P, j_e % P
        order = np.argsort(dest, kind="stable")
        ds = dest[order]
        start = np.concatenate([[0], np.cumsum(np.bincount(ds, minlength=N))])[ds]
        s_e = np.empty(E, dtype=np.int64)
        s_e[order] = np.arange(E) - start
        wcol = off[t_e] + s_e
        xdup[k_e, p_e, wcol] = xbf[src]
        wdup[k_e, p_e, wcol] = w_pad[src]
        return xdup, wdup

    xdupA, wdupA = build_dup(col, row, posA, offA, WA, w_out_pad, KA)
    xdupB, wdupB = build_dup(row, col, posB, offB, WB, w_in_pad, KB)

    # sigma->pi scatter index list for TxI: sigma cell (p', t') holds node
    # v = permB[k][t'*P+p']; its pi DRAM row is (posA%P)*T + posA//P.
    # dma_scatter_add linear index i -> input cell (i%128, i//128), so the
    # list is t'-major / p'-fast. Wrapped int16 layout: index i at
    # [i%16, i//16], replicated across the 8 Q7 cores. Pads -> dump row NL.
    nip = NIMAX - NIMAX % P
    while NL % nip != 0:
        nip -= P
    ncall = NL // nip
    scatI16 = np.zeros((n_cores, P, NL // 16), dtype=np.int16)
    for k in range(n_cores):
        pkB = permB[k]
        pirow = np.full(NL, NL, dtype=np.int64)
        validB = pkB >= 0
        jA = posA[np.maximum(pkB, 0)]
        pirow[validB] = ((jA % P) * T + jA // P)[validB]
        lst = pirow.astype(np.int16)                    # [NL] in i order
        wrapped = np.concatenate(
            [lst[c * nip:(c + 1) * nip].reshape(-1, 16).T for c in range(ncall)],
            axis=1)                                     # [16, NL/16]
        scatI16[k] = np.tile(wrapped, (8, 1))

    xT = np.zeros((n_cores, CH, NL), dtype=np.float32)
    for k in range(n_cores):
        pk = permA[k]
        valid = pk >= 0
        xg = np.zeros((NL, CH), dtype=np.float32)
        xg[valid] = np.asarray(x, dtype=np.float32)[pk[valid]]
        xT[k] = xg.T

    cfg = dict(
        N=N, E=E, NPC=NPC, T=T, NL=NL, WA=WA, WB=WB, KA=KA, KB=KB,
        n_cores=n_cores, chunksA=chunk_plan(DA), chunksB=chunk_plan(DB),
        nip=nip, ncall=ncall,
    )
    arrays = dict(
        xdupA=xdupA, wdupA=wdupA, xdupB=xdupB, wdupB=wdupB, xT=xT,
        scatI16=scatI16, permA=permA,
    )
    return cfg, arrays


def make_in_maps(cfg, arrays, w_z, b_z, w_h, b_h, lin_w, lin_b):
    """AT row layout: [x^T (0:16) | zeros (16:32) | TxO^T (32:48) | TxI^T (48:64)].
    Wcat rows match; rows 16:32 are zero (contraction-dim padding is free)."""
    n_cores = cfg["n_cores"]
    w_id0 = np.concatenate([w_z[0, 0, :CH], w_h[0, 0, :CH]], axis=1).astype(np.float32)
    w_id1 = np.concatenate([w_z[1, 0, :CH], w_h[1, 0, :CH]], axis=1).astype(np.float32)
    w_dif = np.concatenate(
        [np.concatenate([w_z[0, 1, :CH], w_h[0, 1, :CH]], axis=1),
         np.concatenate([w_z[1, 1, :CH], w_h[1, 1, :CH]], axis=1)],
        axis=0).astype(np.float32)
    bias = np.concatenate([b_z, b_h]).astype(np.float32).reshape(P, 1)
    in_maps = []
    for k in range(n_cores):
        in_maps.append({
            "xT": np.ascontiguousarray(arrays["xT"][k]),
            "xdupA": np.ascontiguousarray(arrays["xdupA"][k]),
            "wdupA": np.ascontiguousarray(arrays["wdupA"][k]),
            "xdupB": np.ascontiguousarray(arrays["xdupB"][k]),
            "wdupB": np.ascontiguousarray(arrays["wdupB"][k]),
            "scatI16": np.ascontiguousarray(arrays["scatI16"][k]),
            "w_id0": w_id0, "w_id1": w_id1, "w_dif": w_dif,
            "bias": bias,
            "lin_w": lin_w.astype(np.float32),
            "lin_b": lin_b.astype(np.float32).reshape(1, 1),
        })
    return in_maps


def postprocess(cfg, arrays, results):
    """results[k]['out'] is [1, NL]; scatter back to [N, 1] full output."""
    N, NL = cfg["N"], cfg["NL"]
    out = np.zeros((N, 1), dtype=np.float32)
    for k in range(cfg["n_cores"]):
        o = np.asarray(results[k]["out"]).reshape(NL)
        pk = arrays["permA"][k]
        valid = pk >= 0
        out[pk[valid], 0] = o[valid]
    return out


# ---------------------------------------------------------------------------
# device kernel
# ---------------------------------------------------------------------------

def build_kernel(cfg, debug=False):
    T, NL, WA, WB = cfg["T"], cfg["NL"], cfg["WA"], cfg["WB"]
    KA, KB = cfg["KA"], cfg["KB"]
    nip, ncall = cfg["nip"], cfg["ncall"]
    f32 = mybir.dt.float32
    bf16 = mybir.dt.bfloat16
    i16 = mybir.dt.int16

    nc = bacc.Bacc()

    xT_p = nc.declare_dram_parameter("xT", [CH, NL], f32, isOutput=False)
    xdupA_p = nc.declare_dram_parameter("xdupA", [P, WA, CH], bf16, isOutput=False)
    wdupA_p = nc.declare_dram_parameter("wdupA", [P, WA, KA], bf16, isOutput=False)
    xdupB_p = nc.declare_dram_parameter("xdupB", [P, WB, CH], bf16, isOutput=False)
    wdupB_p = nc.declare_dram_parameter("wdupB", [P, WB, KB], bf16, isOutput=False)
    scatI_p = nc.declare_dram_parameter("scatI16", [P, NL // 16], i16, isOutput=False)
    w_id0_p = nc.declare_dram_parameter("w_id0", [CH, P], f32, isOutput=False)
    w_id1_p = nc.declare_dram_parameter("w_id1", [CH, P], f32, isOutput=False)
    w_dif_p = nc.declare_dram_parameter("w_dif", [2 * CH, P], f32, isOutput=False)
    bias_p = nc.declare_dram_parameter("bias", [P, 1], f32, isOutput=False)
    lin_w_p = nc.declare_dram_parameter("lin_w", [FILT, 1], f32, isOutput=False)
    lin_b_p = nc.declare_dram_parameter("lin_b", [1, 1], f32, isOutput=False)
    out_p = nc.declare_dram_parameter("out", [1, NL], f32, isOutput=True)

    # TxI pi-ordered bounce: 256B rows (64 f32), channels 0:16 used;
    # row NL is a dump slot for scatter pads
    txip_d = nc.dram_tensor("txip", [NL + 1, 64], f32)

    with ExitStack() as ctx:
        tc = ctx.enter_context(tile.TileContext(nc))
        persist = ctx.enter_context(tc.tile_pool(name="persist", bufs=1))
        wpool = ctx.enter_context(tc.tile_pool(name="wpool", bufs=3))
        xpool = ctx.enter_context(tc.tile_pool(name="xpool", bufs=3))
        spool = ctx.enter_context(tc.tile_pool(name="spool", bufs=2))
        mpool = ctx.enter_context(tc.tile_pool(name="mpool", bufs=2))
        work = ctx.enter_context(tc.tile_pool(name="work", bufs=2))
        psum = ctx.enter_context(tc.tile_pool(name="psum", bufs=2, space="PSUM"))
        psum_pre = ctx.enter_context(tc.tile_pool(name="psum_pre", bufs=2, space="PSUM"))

        # ---- persistent tiles & input DMAs ----
        AT = persist.tile([FILT, NL], f32)
        TxC = persist.tile([P, T, 2 * CH], f32)
        TxIs = persist.tile([P, T, 64], f32)
        scatI_t = persist.tile([P, NL // 16], i16)
        Wcat = persist.tile([FILT, P], f32)
        w_id0_t = persist.tile([CH, P], f32)
        w_id1_t = persist.tile([CH, P], f32)
        bias_t = persist.tile([P, 1], f32)
        bias_h = persist.tile([P, 1], f32)
        lin_w_t = persist.tile([FILT, 1], f32)
        lin_b_t = persist.tile([1, 1], f32)
        ident = persist.tile([P, P], f32)

        nc.gpsimd.memset(AT[0:2 * CH, :], 0.0)
        nc.gpsimd.memset(Wcat[0:2 * CH, :], 0.0)
        # zero TxIs once; its zeroed image also zero-fills the scatter target
        nc.gpsimd.memset(TxIs[:], 0.0)
        nc.sync.dma_start(out=txip_d[0:NL, :], in_=TxIs[:])
        nc.sync.dma_start(out=AT[0:CH, :], in_=xT_p[:])
        nc.sync.dma_start(out=scatI_t[:], in_=scatI_p[:])
        nc.sync.dma_start(out=w_id0_t[:], in_=w_id0_p[:])
        nc.sync.dma_start(out=w_id1_t[:], in_=w_id1_p[:])
        nc.sync.dma_start(out=Wcat[2 * CH:4 * CH, :], in_=w_dif_p[:])
        nc.sync.dma_start(out=bias_t[:], in_=bias_p[:])
        nc.sync.dma_start(out=lin_w_t[:], in_=lin_w_p[:])
        nc.sync.dma_start(out=lin_b_t[:], in_=lin_b_p[:])
        make_identity(nc, ident[:])

        nc.vector.tensor_add(out=Wcat[0:CH, :], in0=w_id0_t[:], in1=w_id1_t[:])
        lin_w_bf = persist.tile([FILT, 1], bf16)
        nc.scalar.copy(out=lin_w_bf[:], in_=lin_w_t[:])
        # bias halves: Z-part scaled by 0.5 for the tanh-based sigmoid
        nc.vector.tensor_scalar_mul(out=bias_h[0:FILT, :], in0=bias_t[0:FILT, :],
                                    scalar1=0.5)
        nc.vector.tensor_copy(out=bias_h[FILT:P, :], in_=bias_t[FILT:P, :])

        # ---- streamed message passing ----
        # per chunk: load [P, Wc, K] weights + [P, Wc, CH] features; on-chip
        # deg = reduce(w) (two bf16 2x-mode folds K->K/3, then reduce),
        # s = 1/deg, m = x * s (bf16), segment-reduce into the target.
        # B first: its sigma->pi permute then overlaps the A stream.
        ctx.enter_context(nc.allow_low_precision(
            reason="bf16 edge pipeline; rel tolerance 2e-2 vs bf16 ~4e-3"))

        def stream(xdup_p, wdup_p, Kd, chunks, tx_out, ch0, after_chunk=None):
            K3 = Kd // 3
            for (t0, t1, o0, o1, runs) in chunks:
                Wc = o1 - o0
                wd = wpool.tile([P, CWMAX, Kd], bf16, tag="wd")
                nc.sync.dma_start(out=wd[:, 0:Wc, :], in_=wdup_p[:, o0:o1, :])
                xd = xpool.tile([P, CWMAX, CH], bf16, tag="xd")
                nc.scalar.dma_start(out=xd[:, 0:Wc, :], in_=xdup_p[:, o0:o1, :])
                fold = mpool.tile([P, CWMAX, K3], bf16, tag="fold")
                nc.vector.tensor_tensor(out=fold[:, 0:Wc, :],
                                        in0=wd[:, 0:Wc, 0:K3],
                                        in1=wd[:, 0:Wc, K3:2 * K3],
                                        op=mybir.AluOpType.add)
                nc.vector.tensor_tensor(out=fold[:, 0:Wc, :],
                                        in0=fold[:, 0:Wc, :],
                                        in1=wd[:, 0:Wc, 2 * K3:Kd],
                                        op=mybir.AluOpType.add)
                K9 = K3 // 3
                fold2 = mpool.tile([P, CWMAX, K9], bf16, tag="fold2")
                nc.vector.tensor_tensor(out=fold2[:, 0:Wc, :],
                                        in0=fold[:, 0:Wc, 0:K9],
                                        in1=fold[:, 0:Wc, K9:2 * K9],
                                        op=mybir.AluOpType.add)
                nc.vector.tensor_tensor(out=fold2[:, 0:Wc, :],
                                        in0=fold2[:, 0:Wc, :],
                                        in1=fold[:, 0:Wc, 2 * K9:K3],
                                        op=mybir.AluOpType.add)
                s = spool.tile([P, CWMAX], f32, tag="s")
                nc.vector.tensor_reduce(out=s[:, 0:Wc], in_=fold2[:, 0:Wc, :],
                                        axis=mybir.AxisListType.X,
                                        op=mybir.AluOpType.add)
                nc.vector.tensor_scalar_max(out=s[:, 0:Wc], in0=s[:, 0:Wc],
                                            scalar1=1e-30)
                sb = spool.tile([P, CWMAX], bf16, tag="sb")
                nc.vector.reciprocal(out=sb[:, 0:Wc], in_=s[:, 0:Wc])
                m = mpool.tile([P, CWMAX, CH], bf16, tag="m")
                nc.vector.tensor_tensor(out=m[:, 0:Wc, :], in0=xd[:, 0:Wc, :],
                                        in1=sb[:, 0:Wc].to_broadcast([P, Wc, CH]),
                                        op=mybir.AluOpType.mult)
                ro = 0
                for (r0, r1, D) in runs:
                    nt = r1 - r0
                    nc.vector.tensor_reduce(
                        out=tx_out[:, r0:r1, ch0:ch0 + CH],
                        in_=m[:, ro:ro + nt * D, :].rearrange(
                            "p (t d) c -> p t c d", t=nt),
                        axis=mybir.AxisListType.X, op=mybir.AluOpType.add)
                    ro += nt * D
                if after_chunk is not None:
                    after_chunk(t1)

        # sigma->pi transport of TxI: as soon as a 7-tile group of TxIs is
        # reduced, scatter its node rows into pi-ordered DRAM (GpSimd SWDGE,
        # fully overlapped under stream B). One strided read-back afterwards.
        tpg = nip // P
        scat_state = {"next": 0}

        def emit_scatters(t_done):
            while (scat_state["next"] < ncall
                   and (scat_state["next"] + 1) * tpg <= t_done):
                c = scat_state["next"]
                nc.gpsimd.dma_scatter_add(
                    txip_d[:], TxIs[:, c * tpg:(c + 1) * tpg, :],
                    scatI_t[:, c * (nip // 16):(c + 1) * (nip // 16)],
                    nip, nip, 64)
                scat_state["next"] += 1

        stream(xdupB_p, wdupB_p, KB, cfg["chunksB"], TxIs, 0,
               after_chunk=emit_scatters)
        emit_scatters(T)
        # read back pi-ordered TxI into TxC channels 16:32 (SWDGE fast path:
        # plain strided DMA, keeps the sync/scalar queues free for A loads)
        nc.gpsimd.dma_start(
            out=TxC[:, :, CH:2 * CH],
            in_=txip_d[0:NL, 0:CH].rearrange("(p t) c -> p t c", p=P))

        stream(xdupA_p, wdupA_p, KA, cfg["chunksA"], TxC, 0)

        # ---- transposes into AT rows 32:64 ----
        # 4 tiles per transpose: out rows 32*i:32*i+32 = tile (g0+i) [TxO|TxI]
        for g0 in range(0, T, 4):
            nt = min(4, T - g0)
            ps = psum.tile([P, P], f32, tag="tps")
            nc.tensor.transpose(
                out=ps[0:nt * 2 * CH, :],
                in_=TxC[:, g0:g0 + nt, :].rearrange("p t c -> p (t c)"),
                identity=ident[:])
            for i in range(nt):
                nc.scalar.copy(
                    out=AT[2 * CH:4 * CH, (g0 + i) * P:(g0 + i + 1) * P],
                    in_=ps[i * 2 * CH:(i + 1) * 2 * CH, :])

        # ---- epilogue ----
        out_sb = persist.tile([1, NL], f32)
        CW = 512
        nchunks = (NL + CW - 1) // CW
        for c in range(nchunks):
            lo = c * CW
            w = min(CW, NL - lo)
            pre = psum_pre.tile([P, CW], f32, tag="pre")
            nc.tensor.matmul(out=pre[:, 0:w], lhsT=Wcat[:], rhs=AT[:, lo:lo + w],
                             start=True, stop=True)
            z = work.tile([FILT, CW], f32, tag="z")
            ht = work.tile([FILT, CW], f32, tag="ht")
            # h = relu((1-sigmoid(zpre))*tanh(hpre)) = relu(ht*(0.5-0.5*t))
            # with t = tanh(0.5*zpre + 0.5*b_z), ht = tanh(hpre + b_h)
            nc.scalar.activation(out=z[:, 0:w], in_=pre[0:FILT, 0:w],
                                 func=mybir.ActivationFunctionType.Tanh,
                                 bias=bias_h[0:FILT, :], scale=0.5)
            nc.scalar.activation(out=ht[:, 0:w], in_=pre[FILT:P, 0:w],
                                 func=mybir.ActivationFunctionType.Tanh,
                                 bias=bias_h[FILT:P, :], scale=1.0)
            nc.vector.tensor_scalar(out=z[:, 0:w], in0=z[:, 0:w],
                                    scalar1=-0.5, scalar2=0.5,
                                    op0=mybir.AluOpType.mult,
                                    op1=mybir.AluOpType.add)
            h = work.tile([FILT, CW], f32, tag="h")
            nc.vector.tensor_mul(out=h[:, 0:w], in0=z[:, 0:w], in1=ht[:, 0:w])
            nc.vector.tensor_scalar_max(out=h[:, 0:w], in0=h[:, 0:w], scalar1=0.0)
            ps2 = psum.tile([1, CW], f32, tag="ps2")
            nc.tensor.matmul(out=ps2[:, 0:w], lhsT=lin_w_t[:], rhs=h[:, 0:w],
                             start=True, stop=True)
            nc.vector.tensor_scalar_add(out=out_sb[:, lo:lo + w], in0=ps2[:, 0:w],
                                        scalar1=lin_b_t[0:1, :])
        nc.sync.dma_start(out=out_p[:], in_=out_sb[:])

    nc.compile()
    return nc


# ---------------------------------------------------------------------------
# harness entry point
# ---------------------------------------------------------------------------

_CACHE = {}


def kernel(x, edge_index, edge_weight, w_z, b_z, w_r, b_r, w_h, b_h, lin_w, lin_b):
    """Distributed DCRNN forward on 8 TRN2 NeuronCores.

    Takes full unsharded inputs, returns the full [N, 1] float32 output.
    (w_r/b_r are dead inputs: H0 = 0 makes the reset gate a no-op.)
    """
    from concourse.bass_utils import run_bass_kernel_spmd

    x = np.ascontiguousarray(np.asarray(x, dtype=np.float32))
    cfg, arrays = preprocess(x, np.asarray(edge_index), np.asarray(edge_weight),
                             n_cores=8)
    in_maps = make_in_maps(cfg, arrays, np.asarray(w_z, np.float32),
                           np.asarray(b_z, np.float32),
                           np.asarray(w_h, np.float32),
                           np.asarray(b_h, np.float32),
                           np.asarray(lin_w, np.float32),
                           np.asarray(lin_b, np.float32))
    key = (cfg["N"], cfg["E"], cfg["WA"], cfg["WB"], cfg["KA"], cfg["KB"],
           tuple(tuple(c[:4]) for c in cfg["chunksA"]),
           tuple(tuple(c[:4]) for c in cfg["chunksB"]))
    nc = _CACHE.get(key)
    if nc is None:
        nc = build_kernel(cfg)
        _CACHE[key] = nc
    res = run_bass_kernel_spmd(nc, in_maps, core_ids=list(range(8)))
    return postprocess(cfg, arrays, res.results)
